# revision 15
# baseline (speedup 1.0000x reference)
"""Trainium2 Bass kernel for nn_AttentionHead_51178830299302.

Single attention head: B=8, S=2048, E=1024, H=64, fp32 I/O, decoder
(causal) masking plus a pad-pad coupling term (padded queries attend
bidirectionally to padded keys).

Strategy:
  * Data-parallel over batch: one batch element per NeuronCore (8 cores).
  * Host-side, each sequence is stably partitioned into [valid | pad]
    positions (order preserved).  The masked softmax then decomposes
    exactly into two independent attention problems:
      - valid x valid with plain causal masking,
      - pad x pad with full bidirectional softmax (no mask),
    which skips ~60% of the S x S exp/matmul work vs. the dense mask.
  * bf16 matmul pipeline (fp32 PSUM accumulation), exp on ScalarE
    straight from PSUM.
  * All matmuls are N=512-wide streams: q/k/v projections produce
    transposed [head, seq] layouts; V is moved to its natural layout
    with one XBAR transpose DMA; attention output is produced
    transposed [H+1, seq] with the softmax row-sum riding along as an
    appended ones-row of V; the final divide+transpose happens on host.
  * Slot-padded keys are killed via one augmented contraction row
    (score += kill_j * NEG); causal masking inside diagonal 128-blocks
    is one bf16 multiply with a constant 0/1 tril tile.

kernel(**inputs) takes the FULL unsharded fp32 inputs and returns the
FULL [8, 2048, 64] fp32 output.
"""

import numpy as np
import ml_dtypes

B, S, E, H = 8, 2048, 1024, 64
NEG = -100000.0
P = 128
BF = ml_dtypes.bfloat16

_NC_CACHE: dict = {}


def _patch_tile_drain():
    """The stock TileContext exit hangs every global-clock wait on a single
    Drain instruction; this container's walrus caps sync waits at 1 per
    instruction.  Split the waits across single-wait nops, and drop the
    second (post-semclear) all-engine barrier — engines halt right after,
    and NEFF re-execution only starts once every engine has halted."""
    import concourse.tile as tile
    import concourse.mybir as mybir
    from bass_rust import ScopedClock

    if getattr(tile.TileContext, "_drain_waits_split", False):
        return

    def _drain_and_barrier(self, tick_clock, wait_clock):
        nc = self.nc
        carrier = nc.sync.nop(nofuse=True)
        wait_clock.add_sem_waits(
            carrier.ins, ScopedClock({None: tick_clock.global_clock})
        )
        si = carrier.ins.sync_info
        waits = list(si.on_wait) if si and si.on_wait else []
        if len(waits) > 1:
            si.on_wait = waits[:1]
            for w in waits[1:]:
                n = nc.sync.nop(nofuse=True)
                nsi = n.ins.sync_info
                if nsi is None:
                    n.ins.sync_info = mybir.SyncInfo(on_wait=[w], on_update=[])
                else:
                    nsi.on_wait = [w]
        nc.sync.drain()
        nc.all_engine_barrier()
        popped = nc._tile_sem_poison_stack.pop()
        assert popped is self._sem_poison
        nc.clear_and_free_semaphores(list(self.sems.allocated().values()))

    tile.TileContext._drain_and_barrier = _drain_and_barrier
    tile.TileContext._drain_waits_split = True


def _patch_sync_wait_split():
    """This container's walrus codegen rejects instructions carrying more
    than one sync wait.  Post-process the serialized BIR: hoist excess
    waits onto injected NoOps on the same engine, just before the
    instruction (the sequencer executes them in order, so semantics are
    preserved)."""
    import json
    import concourse.bass as bass

    if getattr(bass.Bass, "_sync_wait_split", False):
        return
    orig = bass.Bass.to_json_bytes

    def to_json_bytes(self) -> bytes:
        j = json.loads(orig(self))
        ctr = [0]

        def fix_block(blk):
            insts = blk.get("instructions")
            if not isinstance(insts, list):
                return
            out = []
            for inst in insts:
                si = inst.get("sync_info")
                ow = (si or {}).get("on_wait") or []
                if len(ow) > 1:
                    si["on_wait"] = ow[-1:]
                    for w in ow[:-1]:
                        ctr[0] += 1
                        out.append(
                            {
                                "debug": inst.get("debug", 0),
                                "engine": inst["engine"],
                                "ins": [],
                                "name": f"I-wsplit-{ctr[0]}",
                                "opcode": "NoOp",
                                "outs": [],
                                "sync_info": {"on_wait": [w], "on_update": []},
                            }
                        )
                out.append(inst)
            blk["instructions"] = out

        def rec(o):
            if isinstance(o, dict):
                if "instructions" in o:
                    fix_block(o)
                for v in o.values():
                    rec(v)
            elif isinstance(o, list):
                for v in o:
                    rec(v)

        rec(j)
        return json.dumps(j).encode()

    bass.Bass.to_json_bytes = to_json_bytes
    bass.Bass._sync_wait_split = True


def build_nc(SV: int, SP: int):
    """Build the SPMD per-core Bass program.

    Per-core DRAM tensors:
      hsT  [E, SVP]   bf16   sorted hidden state, transposed (E-major)
      wqk  [E, 128]   bf16   [Wq/sqrt(H) | Wk]
      wv   [E, H]     bf16
      bqk  [128, 1]   f32    [bq/sqrt(H) ; bk]
      kill [1, SVP]   bf16   1.0 on slot-padding positions
      c01  [128,1024] bf16   tril keep-mask: c01[j, 512+y] = (j <= y)
      outT [65, SVP]  f32    rows 0..63 unnormalized output^T, row 64
                             softmax denominators (host divides)
    """
    import concourse.bass as bass
    import concourse.mybir as mybir
    import concourse.tile as tile
    from contextlib import ExitStack

    _patch_tile_drain()
    _patch_sync_wait_split()
    bf, f32 = mybir.dt.bfloat16, mybir.dt.float32
    Exp = mybir.ActivationFunctionType.Exp

    SVP = SV + SP
    NKC_V, NKC_P = SV // P, SP // P
    NT = SVP // P

    nc = bass.Bass("TRN2", target_bir_lowering=False, debug=False)
    NSL = (SVP + 511) // 512  # 512-col projection slices
    # hsT packed slice-major: [128, NSL, 8, 512]; per partition each slice
    # is one contiguous 8 KiB run -> 128 maximal DMA descriptors per slice.
    hsT_d = nc.dram_tensor("hsT", [P, NSL, 8, 512], bf, kind="ExternalInput").ap()
    wqk_d = nc.dram_tensor("wqk", [P, 8, P], bf, kind="ExternalInput").ap()
    wv_d = nc.dram_tensor("wv", [P, 8, H], bf, kind="ExternalInput").ap()
    bqk_d = nc.dram_tensor("bqk", [P, 1], f32, kind="ExternalInput").ap()
    kill_d = nc.dram_tensor("kill", [2, SVP], bf, kind="ExternalInput").ap()
    c01_d = nc.dram_tensor("c01", [P, 1024], bf, kind="ExternalInput").ap()
    outT_d = nc.dram_tensor("outT", [H + 1, SVP], f32, kind="ExternalOutput").ap()

    with tile.TileContext(nc) as tc, ExitStack() as ctx:
        singles = ctx.enter_context(tc.tile_pool(name="singles", bufs=1))

        # PE warm-up source tile: memset first so the warm-up matmul
        # stream starts as soon as the engines come up.
        wz = singles.tile([P, 256], bf)
        nc.vector.memset(wz[:], 0.0)

        wqk_s = singles.tile([P, 8, P], bf)
        wv_s = singles.tile([P, 8, H], bf)
        bqk_s = singles.tile([P, 1], f32)
        c01_s = singles.tile([P, 1024], bf)

        # qT/kT: 64 head rows + 1 augmented mask row (row 64).
        # score += qT_aug[64] * kT_aug[64] = NEG * kill_j
        qT = singles.tile([65, SVP], bf)
        kT = singles.tile([65, SVP], bf)

        # V in natural [seq-part, head] layout with an appended ones
        # column (row-sums of the attention weights ride along in the
        # AV matmul as output row H).
        vS = singles.tile([P, NT, H + 1], bf)
        nc.vector.memset(vS[:, :, H : H + 1], 1.0)
        vT = singles.tile([H, SVP], bf)
        # XBAR transpose needs a contiguous destination on HW; stage here,
        # then strided-copy into vS (which carries the ones column).
        vN = singles.tile([P, NT, H], bf)

        # hidden state, E-major.  Two segments per 128-chunk: a small
        # head segment so the first projection slice starts early, the
        # rest as one large transfer.  Alternate between the two HWDGE
        # rings (SP + Activation) for trigger-throughput.
        hsT = singles.tile([P, NSL, 8, 512], bf)
        # sync ring: even hsT slices (slice 0 first).  scalar ring: the
        # small weight/constant tensors first (needed by the very first
        # matmuls), then odd hsT slices.
        nc.sync.dma_start(hsT[:, 0, :, :], hsT_d[:, 0, :, :])
        nc.scalar.dma_start(wqk_s[:], wqk_d)
        nc.scalar.dma_start(wv_s[:], wv_d)
        nc.scalar.dma_start(bqk_s[:], bqk_d)
        nc.scalar.dma_start(kT[64:65, :], kill_d[0:1, :])
        nc.scalar.dma_start(qT[64:65, :], kill_d[1:2, :])
        nc.scalar.dma_start(c01_s[:], c01_d)
        for si in range(1, NSL):
            eng = nc.scalar if si % 2 == 1 else nc.sync
            eng.dma_start(hsT[:, si, :, :], hsT_d[:, si, :, :])

        # ------- interleaved projections + attention -------
        # Emission order interleaves projection slices with attention
        # q-blocks whose inputs are already covered, so the PE stream has
        # no phase barrier and HAM stays warm.
        with tc.tile_pool(name="pp", bufs=1, space="PSUM") as pp, \
             tc.tile_pool(name="acc", bufs=2, space="PSUM") as acc, \
             tc.tile_pool(name="spsum", bufs=2, space="PSUM") as spsum, \
             tc.tile_pool(name="kstage", bufs=2) as kstage_pool, \
             tc.tile_pool(name="wpool", bufs=4) as wpool, \
             tc.tile_pool(name="opool", bufs=2) as opool, \
             tc.tile_pool(name="warmp", bufs=1, space="PSUM") as warmp:

            warm_ps = warmp.tile([P, 256], f32)
            for _ in range(42):
                nc.tensor.matmul(
                    warm_ps[:], lhsT=wz[:, 0:P], rhs=wz[:], start=True, stop=True
                )

            def emit_proj_slice(sb):
                si = sb // 512
                w = min(512, SVP - sb)
                ps = pp.tile([P, 512], f32)
                for c in range(8):
                    nc.tensor.matmul(
                        ps[:, :w],
                        lhsT=wqk_s[:, c, :],
                        rhs=hsT[:, si, c, :w],
                        start=(c == 0),
                        stop=(c == 7),
                    )
                # q lands on partitions 0:64 -> evacuate straight into qT
                nc.vector.tensor_scalar_add(
                    qT[0:64, sb : sb + w], ps[0:64, :w], bqk_s[0:64, 0:1]
                )
                # k lands on partitions 64:128; engines cannot shift
                # partitions, so stage and bounce via DMA to rows 0:64.
                kst = kstage_pool.tile([P, 512], bf)
                nc.vector.tensor_scalar_add(
                    kst[64:128, :w], ps[64:128, :w], bqk_s[64:128, 0:1]
                )
                nc.scalar.dma_start(kT[0:64, sb : sb + w], kst[64:128, :w])

                pv = acc.tile([H + 1, 512], f32, tag="acc", name="pv")[0:H, :]
                for c in range(8):
                    nc.tensor.matmul(
                        pv[:, :w],
                        lhsT=wv_s[:, c, :],
                        rhs=hsT[:, si, c, :w],
                        start=(c == 0),
                        stop=(c == 7),
                    )
                nc.vector.tensor_copy(vT[:, sb : sb + w], pv[:, :w])
                # vT [64, seq] -> vS [128, tile, 64] via XBAR transpose
                # (contiguous staging dst, then strided copy for ones col)
                ta, tb = sb // P, (sb + w) // P
                nc.sync.dma_start_transpose(vN[:, ta:tb, :], vT[:, sb : sb + w])
                nc.vector.tensor_copy(vS[:, ta:tb, 0:H], vN[:, ta:tb, :])

            def emit_qblock(part, q0r):
                part_q0 = 0 if part == 0 else SV
                part_len = SV if part == 0 else SP
                kc_base = 0 if part == 0 else NKC_V
                w = min(512, part_len - q0r)
                q0 = part_q0 + q0r
                if part == 0:
                    kcs = list(range(0, (q0r + w - 1) // P + 1))
                else:
                    kcs = list(range(NKC_P))

                ot = acc.tile([H + 1, 512], f32, tag="acc", name="ot")
                spb = 512 // w  # score slots per PSUM bank
                cap = 2 * spb  # slots per 2-bank score group
                groups = [kcs[i : i + cap] for i in range(0, len(kcs), cap)]
                n_kc = len(kcs)
                ki = 0
                for grp in groups:
                    st_ps = spsum.tile([P, 2 * 512], f32)
                    wt = wpool.tile([P, 2 * 512], bf)
                    offs = [
                        (i // spb) * 512 + (i % spb) * w for i in range(len(grp))
                    ]
                    for i, kcr in enumerate(grp):
                        kc = kc_base + kcr
                        nc.tensor.matmul(
                            st_ps[:, offs[i] : offs[i] + w],
                            lhsT=kT[0:65, kc * P : (kc + 1) * P],
                            rhs=qT[0:65, q0 : q0 + w],
                            start=True,
                            stop=True,
                        )
                    if 512 % w == 0:  # slots are contiguous
                        n = len(grp) * w
                        nc.scalar.activation(wt[:, 0:n], st_ps[:, 0:n], Exp)
                    else:
                        for off in offs:
                            nc.scalar.activation(
                                wt[:, off : off + w], st_ps[:, off : off + w], Exp
                            )
                    if part == 0:
                        for i, kcr in enumerate(grp):
                            d = kcr * P - q0r
                            if d >= 0:  # diagonal-band block
                                off = offs[i]
                                nc.vector.tensor_mul(
                                    wt[:, off : off + w],
                                    wt[:, off : off + w],
                                    c01_s[:, 512 - d : 512 - d + w],
                                )
                    for i, kcr in enumerate(grp):
                        kc = kc_base + kcr
                        nc.tensor.matmul(
                            ot[:, :w],
                            lhsT=vS[:, kc, :],
                            rhs=wt[:, offs[i] : offs[i] + w],
                            start=(ki == 0),
                            stop=(ki == n_kc - 1),
                        )
                        ki += 1

                osb = opool.tile([H + 1, 512], f32)
                nc.vector.tensor_copy(osb[:, :w], ot[:, :w])
                nc.sync.dma_start(outT_d[:, q0 : q0 + w], osb[:, :w])

            # schedule: proj slice i covers seq cols [512i, 512i+512);
            # a q-block may be emitted once the slices covering both its
            # queries and its keys (and V tiles) have been emitted.
            n_slices = (SVP + 511) // 512
            qblocks = []  # (part, q0r, need_cols)
            for part in range(2):
                part_q0 = 0 if part == 0 else SV
                part_len = SV if part == 0 else SP
                for q0r in range(0, part_len, 512):
                    w = min(512, part_len - q0r)
                    if part == 0:
                        kmax = ((q0r + w - 1) // P + 1) * P
                    else:
                        kmax = SV + SP
                    need = max(part_q0 + q0r + w, kmax)
                    qblocks.append((part, q0r, need))
            qi = 0
            for si in range(n_slices):
                emit_proj_slice(si * 512)
                covered = min((si + 1) * 512, SVP)
                while qi < len(qblocks) and qblocks[qi][2] <= covered:
                    emit_qblock(qblocks[qi][0], qblocks[qi][1])
                    qi += 1
            while qi < len(qblocks):
                emit_qblock(qblocks[qi][0], qblocks[qi][1])
                qi += 1
    return nc


def _prepare(hidden_state, attention_masks, Wq, bq, Wk, bk, Wv, bv):
    """Host-side shard prep: sort each sequence into [valid | pad],
    pad both groups to shared multiples of 128, cast to bf16."""
    hs = np.asarray(hidden_state, dtype=np.float32)
    m = np.asarray(attention_masks)
    perms, nvs = [], []
    for b in range(B):
        mb = np.asarray(m[b]).astype(np.int64)
        perms.append(np.argsort(1 - mb, kind="stable"))
        nvs.append(int(mb.sum()))
    nps = [S - nv for nv in nvs]
    SV = max(128, -(-max(nvs) // P) * P)
    SPn = max(128, -(-max(nps) // P) * P)
    SVP = SV + SPn

    wqk = np.ascontiguousarray(
        np.concatenate(
            [np.asarray(Wq, np.float32) / np.sqrt(H), np.asarray(Wk, np.float32)],
            axis=1,
        ).reshape(8, P, P).transpose(1, 0, 2)
    ).astype(BF)  # [p, c, m]
    wv = np.ascontiguousarray(
        np.asarray(Wv, np.float32).reshape(8, P, H).transpose(1, 0, 2)
    ).astype(BF)  # [p, c, m]
    bqk = np.concatenate(
        [np.asarray(bq, np.float32) / np.sqrt(H), np.asarray(bk, np.float32)]
    ).reshape(P, 1).astype(np.float32)

    # c01[j, 512+y] = 1.0 iff j <= y   (keep when q_rel - d >= j)
    y = np.arange(1024) - 512
    c01 = (np.arange(P)[:, None] <= y[None, :]).astype(BF)

    in_maps = []
    for b in range(B):
        nv, npd = nvs[b], nps[b]
        NSL = (SVP + 511) // 512
        hs_sorted = np.zeros((NSL * 512, E), np.float32)
        hs_sorted[:nv] = hs[b][perms[b][:nv]]
        hs_sorted[SV : SV + npd] = hs[b][perms[b][nv:]]
        # pack [128, NSL, 8, 512]: hsT[p, si, c, j] = hs_sorted[si*512+j, c*128+p]
        hsT = np.ascontiguousarray(
            hs_sorted.reshape(NSL, 512, 8, P).transpose(3, 0, 2, 1)
        ).astype(BF)
        kill = np.zeros((2, SVP), np.float32)
        kill[0, nv:SV] = 1.0
        kill[0, SV + npd :] = 1.0
        kill[1, :] = NEG
        in_maps.append(
            {
                "hsT": hsT,
                "wqk": wqk,
                "wv": wv,
                "bqk": bqk,
                "kill": kill.astype(BF),
                "c01": c01,
            }
        )
    return in_maps, perms, nvs, SV, SPn


def _run(inputs: dict, trace: bool = False):
    from concourse import bass_utils

    in_maps, perms, nvs, SV, SPn = _prepare(**inputs)
    key = (SV, SPn)
    if key not in _NC_CACHE:
        _NC_CACHE[key] = build_nc(SV, SPn)
    nc = _NC_CACHE[key]

    res = bass_utils.run_bass_kernel_spmd(
        nc, in_maps, core_ids=list(range(8)), trace=trace
    )

    bv = np.asarray(inputs["bv"], np.float32)
    out = np.empty((B, S, H), np.float32)
    for b in range(B):
        ot = res.results[b]["outT"]  # [65, SVP]
        dev = (ot[:H] / ot[H]).T  # normalized, [SVP, H]
        nv = nvs[b]
        out[b][perms[b][:nv]] = dev[:nv]
        out[b][perms[b][nv:]] = dev[SV : SV + (S - nv)]
    out += bv  # v-projection bias commutes with the softmax average
    return out, res


def kernel(**inputs) -> np.ndarray:
    out, _ = _run(inputs, trace=False)
    return out
